# revision 10
# baseline (speedup 1.0000x reference)
"""Trainium2 Bass kernel for nn_ConvThreshold: 5x5 scale-adaptive Gaussian
blur (per-pixel bandwidth) + ReLU front + threshold mask.

conv[p] = sum_{dy,dx in [-2,2]} relu(x)[p+(dy,dx)] * t[p]^(dy^2+dx^2)
with t[p] = exp(-1/(2*scale[p]^2 + eps)); mask = conv >= 0.5.

Decomposition: 25 taps grouped into 6 rings by r2 = dy^2+dx^2 in
{0,1,2,4,5,8}; conv = R0 + sum_k t^k * Rk. Ring sums on the TensorEngine
(fp16 1cyc/col matmuls, fp32 PSUM accumulate): vertical taps via banded
lhsT (+K=4 halo matmuls), horizontal taps via free-dim-shifted identity
matmuls. Diagonal rings from free-axis shifts of V1/V2 on DVE/GPSIMD.
Weights: t^1,t^4 via ScalarE exp, t^2/t^5/t^8 derived by fp16 mults.
Products on DVE, final 6-term sum back on TensorE, mask on GPSIMD.

Sharding: 8 cores = 4 images x 2 vertical halves (384 rows, +-2 halo).
Inputs packed host-side into one interleaved [388,1536] tensor (xin|sin)
plus a [12,768] halo pack; outputs packed [384,1536] (conv|mask) - one
DMA per stripe per direction (HWDGE sequencer occupancy ~1.9us/call).
"""

import sys

sys.path.insert(0, "/opt/trn_rl_repo")

from contextlib import ExitStack

import numpy as np

import concourse.bass as bass
import concourse.tile as tile
from concourse import bacc, mybir
from concourse.bass_utils import run_bass_kernel_spmd

F32 = mybir.dt.float32
F16 = mybir.dt.float16

B, H, W = 4, 768, 768
NCORES = 8
SLAB = H // 2
NSTRIPES = SLAB // 128
PAD = 2
WP = W + 2 * PAD  # 772
COLH = [(0, 512), (512, 256)]
RINGS = [1, 2, 4, 5, 8]

_CACHE = {}


def _consts():
    ident = np.eye(128, dtype=np.float16)
    b1 = np.zeros((128, 128), dtype=np.float16)
    b2 = np.zeros((128, 128), dtype=np.float16)
    for m in range(128):
        for d in (-1, 1):
            if 0 <= m + d < 128:
                b1[m + d, m] = 1.0
        for d in (-2, 2):
            if 0 <= m + d < 128:
                b2[m + d, m] = 1.0
    hv1 = np.zeros((4, 128), dtype=np.float16)
    hv1[1, 0] = 1.0
    hv1[2, 127] = 1.0
    hv2 = np.zeros((4, 128), dtype=np.float16)
    hv2[0, 0] = 1.0
    hv2[1, 1] = 1.0
    hv2[2, 126] = 1.0
    hv2[3, 127] = 1.0
    wpack = np.concatenate([ident, b1, b2], axis=1)  # [128, 384]
    hvpack = np.concatenate([hv1, hv2], axis=1)  # [4, 256]
    return {"wpack": wpack, "hvpack": hvpack}


def _build(repeat: int = 1, sb_bufs: int = 3):
    nc = bacc.Bacc(
        "TRN2",
        target_bir_lowering=False,
        debug=False,
        enable_asserts=True,
        num_devices=NCORES,
    )
    ins_d = nc.dram_tensor("ins", [SLAB + 4, 2 * W], F32, kind="ExternalInput").ap()
    hin_d = nc.dram_tensor("hin", [4 * NSTRIPES, W], F32, kind="ExternalInput").ap()
    cd = {
        name: nc.dram_tensor(name, list(arr.shape), F16, kind="ExternalInput").ap()
        for name, arr in _consts().items()
    }
    out_d = nc.dram_tensor("out", [SLAB, 2 * W], F32, kind="ExternalOutput").ap()

    with tile.TileContext(nc, trace_sim=False) as tc, ExitStack() as ctx:
        sb = ctx.enter_context(tc.tile_pool(name="sb", bufs=sb_bufs))
        cb = ctx.enter_context(tc.tile_pool(name="cb", bufs=1))
        ps = ctx.enter_context(tc.tile_pool(name="ps", bufs=1, space="PSUM"))

        wp = cb.tile([128, 384], F16, tag="wpack")
        nc.sync.dma_start(wp[:], cd["wpack"][:])
        hvp = cb.tile([4, 256], F16, tag="hvpack")
        nc.sync.dma_start(hvp[:], cd["hvpack"][:])
        w_id, w_b1, w_b2 = wp[:, 0:128], wp[:, 128:256], wp[:, 256:384]
        w_hv1, w_hv2 = hvp[:, 0:128], hvp[:, 128:256]

        def _body():
          for st in range(NSTRIPES):
            r0 = 128 * st

            insf = sb.tile([128, 2 * W], F32, tag="insf")
            nc.sync.dma_start(insf[:], ins_d[r0 + 2 : r0 + 130, :])
            xf, sf = insf[:, 0:W], insf[:, W : 2 * W]
            hf = sb.tile([4, W], F32, tag="hf")
            nc.sync.dma_start(hf[:], hin_d[4 * st : 4 * st + 4, :])

            x16 = sb.tile([128, WP], F16, tag="x16")
            nc.gpsimd.memset(x16[:, 0:2], 0.0)
            nc.gpsimd.memset(x16[:, WP - 2 : WP], 0.0)
            nc.scalar.activation(
                x16[:, 2 : 2 + W], xf, mybir.ActivationFunctionType.Relu
            )
            h16 = sb.tile([4, WP], F16, tag="h16")
            nc.gpsimd.memset(h16[:, 0:2], 0.0)
            nc.gpsimd.memset(h16[:, WP - 2 : WP], 0.0)
            nc.scalar.activation(
                h16[:, 2 : 2 + W], hf[:], mybir.ActivationFunctionType.Relu
            )

            # weights: t = exp(-1/(2 s^2)); only t^1, t^4 via ACT exp
            u = sb.tile([128, W], F32, tag="u")
            nc.gpsimd.tensor_mul(u[:], sf, sf)  # s^2; the 2x folds into scale
            v = sb.tile([128, W], F32, tag="v")
            nc.vector.reciprocal_approx_fast(v[:], u[:])
            g = {}
            for k in (1, 4):
                gk = sb.tile([128, W], F16, tag=f"g{k}")
                nc.scalar.activation(
                    gk[:], v[:], mybir.ActivationFunctionType.Exp,
                    scale=-float(k) / 2.0,
                )
                g[k] = gk
            for k, (a, b) in ((2, (1, 1)), (8, (4, 4)), (5, (4, 1))):
                gk = sb.tile([128, W], F16, tag=f"g{k}")
                nc.vector.tensor_mul(gk[:], g[a][:], g[b][:])
                g[k] = gk

            v1s = sb.tile([128, WP], F16, tag="v1s")
            v2s = sb.tile([128, WP], F16, tag="v2s")
            for t_ in (v1s, v2s):
                nc.gpsimd.memset(t_[:, 0:2], 0.0)
                nc.gpsimd.memset(t_[:, WP - 2 : WP], 0.0)

            w1 = sb.tile([128, W], F16, tag="w1")
            w4 = sb.tile([128, W], F16, tag="w4")
            for ci, (c0, n) in enumerate(COLH):
                xc = lambda dx, c0=c0, n=n: x16[:, 2 + c0 + dx : 2 + c0 + dx + n]
                hc = lambda dx, c0=c0, n=n: h16[:, 2 + c0 + dx : 2 + c0 + dx + n]

                v1p = ps.tile([128, n], F32, tag=f"v1p{ci}")
                nc.tensor.matmul(v1p[:], w_b1, xc(0), start=True, stop=False)
                nc.tensor.matmul(v1p[:], w_hv1, hc(0), start=False, stop=True)
                v2p = ps.tile([128, n], F32, tag=f"v2p{ci}")
                nc.tensor.matmul(v2p[:], w_b2, xc(0), start=True, stop=False)
                nc.tensor.matmul(v2p[:], w_hv2, hc(0), start=False, stop=True)

                nc.scalar.copy(v1s[:, 2 + c0 : 2 + c0 + n], v1p[:])
                nc.scalar.copy(v2s[:, 2 + c0 : 2 + c0 + n], v2p[:])

                nc.tensor.matmul(v1p[:], w_id, xc(-1), start=False, stop=False)
                nc.tensor.matmul(v1p[:], w_id, xc(+1), start=False, stop=True)
                nc.tensor.matmul(v2p[:], w_id, xc(-2), start=False, stop=False)
                nc.tensor.matmul(v2p[:], w_id, xc(+2), start=False, stop=True)

                nc.vector.tensor_mul(w1[:, c0 : c0 + n], v1p[:], g[1][:, c0 : c0 + n])
                nc.vector.tensor_mul(w4[:, c0 : c0 + n], v2p[:], g[4][:, c0 : c0 + n])

            r2s = sb.tile([128, W], F16, tag="r2s")
            nc.vector.tensor_add(r2s[:], v1s[:, 1 : 1 + W], v1s[:, 3 : 3 + W])
            r8s = sb.tile([128, W], F16, tag="r8s")
            nc.gpsimd.tensor_add(r8s[:], v2s[:, 0:W], v2s[:, 4 : 4 + W])
            r5a = sb.tile([128, W], F16, tag="r5a")
            nc.vector.tensor_add(r5a[:], v1s[:, 0:W], v1s[:, 4 : 4 + W])
            r5b = sb.tile([128, W], F16, tag="r5b")
            nc.vector.tensor_add(r5b[:], v2s[:, 1 : 1 + W], v2s[:, 3 : 3 + W])
            r5s = sb.tile([128, W], F16, tag="r5s")
            nc.vector.tensor_add(r5s[:], r5a[:], r5b[:])

            w2 = sb.tile([128, W], F16, tag="w2")
            nc.vector.tensor_mul(w2[:], r2s[:], g[2][:])
            w5 = sb.tile([128, W], F16, tag="w5")
            nc.vector.tensor_mul(w5[:], r5s[:], g[5][:])
            w8 = sb.tile([128, W], F16, tag="w8")
            nc.gpsimd.tensor_mul(w8[:], r8s[:], g[8][:])

            om = sb.tile([128, 2 * W], F32, tag="om")  # conv | mask
            for ci, (c0, n) in enumerate(COLH):
                cp = ps.tile([128, n], F32, tag=f"cp{ci}")
                nc.tensor.matmul(
                    cp[:], w_id, x16[:, 2 + c0 : 2 + c0 + n], start=True, stop=False
                )
                for wk in (w1, w2, w4, w5):
                    nc.tensor.matmul(
                        cp[:], w_id, wk[:, c0 : c0 + n], start=False, stop=False
                    )
                nc.tensor.matmul(
                    cp[:], w_id, w8[:, c0 : c0 + n], start=False, stop=True
                )
                nc.scalar.copy(om[:, c0 : c0 + n], cp[:])
                nc.gpsimd.tensor_scalar(
                    om[:, W + c0 : W + c0 + n], om[:, c0 : c0 + n],
                    0.5, None, mybir.AluOpType.is_ge,
                )

            nc.sync.dma_start(out_d[r0 : r0 + 128, :], om[:])

        if repeat == 1:
            _body()
        else:
            with tc.For_i(0, repeat, 1):
                _body()

    nc.compile()
    return nc


def kernel(bev_map: np.ndarray, bev_scale: np.ndarray):
    assert bev_map.shape == (B, 1, H, W) and bev_scale.shape == (B, 1, H, W)
    if "nc" not in _CACHE:
        _CACHE["nc"] = _build()
    nc = _CACHE["nc"]

    consts = _consts()
    in_maps = []
    for c in range(NCORES):
        b, hh = c // 2, c % 2
        padded = np.pad(bev_map[b, 0], ((2, 2), (0, 0)))  # [772, W]
        xin = padded[hh * SLAB : hh * SLAB + SLAB + 4]  # [388, W]
        sin = bev_scale[b, 0, hh * SLAB : (hh + 1) * SLAB]  # [384, W]
        ins = np.zeros((SLAB + 4, 2 * W), dtype=np.float32)
        ins[:, 0:W] = xin
        ins[2 : 2 + SLAB, W : 2 * W] = sin
        hin = np.empty((4 * NSTRIPES, W), dtype=np.float32)
        for st in range(NSTRIPES):
            r0 = 128 * st
            hin[4 * st : 4 * st + 2] = xin[r0 : r0 + 2]
            hin[4 * st + 2 : 4 * st + 4] = xin[r0 + 130 : r0 + 132]
        m = {"ins": ins, "hin": hin}
        m.update({k: v.copy() for k, v in consts.items()})
        in_maps.append(m)

    res = run_bass_kernel_spmd(nc, in_maps, list(range(NCORES))).results

    conv = np.empty((B, 1, H, W), dtype=np.float32)
    mask = np.empty((B, 1, H, W), dtype=np.float32)
    for c in range(NCORES):
        b, hh = c // 2, c % 2
        conv[b, 0, hh * SLAB : (hh + 1) * SLAB] = res[c]["out"][:, 0:W]
        mask[b, 0, hh * SLAB : (hh + 1) * SLAB] = res[c]["out"][:, W : 2 * W]
    return conv, mask
